# revision 73
# baseline (speedup 1.0000x reference)
"""Causal self-attention (B=2, S=2048, E=1024, H=16, D=64) on 8 NeuronCores.

Sharding: core = (batch b, head-group g of 4 heads).  Data parallel on B,
tensor parallel on heads.  Each core computes q/k/v projections for its 4
heads, causal flash attention, and a partial output projection
(att_out @ w_o[group rows]); the host sums the 4 partial outputs per batch.

All operands are bf16 on SBUF (PSUM accumulation stays fp32).  Layouts
(every matmul contraction dim sits on SBUF partitions):
  xT  [E=1024, S=2048]   host-transposed x[b], bf16, per-512-col slice tiles
  qT/kT [128 per head-pair, 512] per slice; head h at rows 64h..64h+63
  v   [sc, 4, 65] per slice; ones column 64 (memset) -> rowsum comes out
                         of the same PSUM accumulation as attn@V
  scores transposed: S^T [sk_chunk=128, sq_block=256] fp32 PSUM,
                     2 heads row-tiled (K=64 at array rows 0-63 / 64-127)
  exp on ScalarE in [128, 1024] batches (2 chunks x 2 heads), scale=1/8
                     fused, output bf16; no max-subtraction (scores < ~3)
  output transposed: oT [E, S] bf16 = w_o^T @ att^T (host re-transposes)

Fully fused single-phase schedule: the attention stream is software-
pipelined (PE emits scores(g+1) before av(g); each block's last av +
normalize are deferred into the next block), and ALL other PE work --
q/k/v projection chains and output-projection units -- is drip-fed into
the attention stream's ACT-bound slack through a work queue.  Projection
units for slice t+1 drip during attention stretch t (flushed at the
boundary); oproj units for slice t drip during stretch t+1.  PE is the
only binding engine (~116us of matmul rows); ACT carries only the exp
stream (~75us), DVE/Pool split the copies/normalize.

PSUM budget (8 banks of 2KB): 2x2 scores + 2x1 attn@V-accum (both head
parities share one zeroed bank, start=False) + 2x1 proj/oproj chain.
"""

import sys

sys.path.insert(0, "/opt/trn_rl_repo")

import numpy as np
import ml_dtypes
from contextlib import ExitStack

import concourse.bass as bass
import concourse.bacc as bacc
import concourse.mybir as mybir
import concourse.tile as tile
from concourse import bass_utils
from concourse import library_config

F32 = mybir.dt.float32
BF16 = mybir.dt.bfloat16
F8 = mybir.dt.float8e4
DR = mybir.MatmulPerfMode.DoubleRow
AF = mybir.ActivationFunctionType

B, S, E, H, D = 2, 2048, 1024, 16, 64
HPC = 4                 # heads per core
DP = HPC * D            # 256 d' columns per core
NCORES = 8
SQ = 256                # query block
CH = 128                # kv chunk
GRP = 2                 # kv chunks per exp batch
EC = E // 128           # 8 e-chunks
NSC = S // 128          # 16 s-chunks


def make_tri():
    # multiplicative causal mask for the diagonal 128x128 square of a
    # [sk,sq] tile: keep sq >= sk
    return (np.arange(128)[None, :] >= np.arange(128)[:, None]).astype(
        ml_dtypes.bfloat16)


def build_kernel(debug=False):
    nc = bacc.Bacc("TRN2", target_bir_lowering=False, debug=False)

    xT_d = nc.dram_tensor("xT", [E, S], BF16, kind="ExternalInput")
    xT8_d = nc.dram_tensor("xT8", [E, S], F8, kind="ExternalInput")
    wq_d = nc.dram_tensor("wq", [E, DP], F8, kind="ExternalInput")
    wk_d = nc.dram_tensor("wk", [E, DP], BF16, kind="ExternalInput")
    wv_d = nc.dram_tensor("wv", [E, DP], BF16, kind="ExternalInput")
    wo_d = nc.dram_tensor("wo", [DP, E], BF16, kind="ExternalInput")
    tri_d = nc.dram_tensor("tri", [128, 128], BF16, kind="ExternalInput")
    oT_d = nc.dram_tensor("oT", [E, S], BF16, kind="ExternalOutput")

    with tile.TileContext(nc) as tc, ExitStack() as ctx:
        dat = ctx.enter_context(tc.tile_pool(name="dat", bufs=1))
        st_pool = ctx.enter_context(tc.tile_pool(name="st", bufs=2, space="PSUM"))
        ou_pool = ctx.enter_context(tc.tile_pool(name="ou", bufs=2, space="PSUM"))
        po_pool = ctx.enter_context(tc.tile_pool(name="po", bufs=2, space="PSUM"))
        pt_pool = ctx.enter_context(tc.tile_pool(name="pt", bufs=8))
        nrm_pool = ctx.enter_context(tc.tile_pool(name="nrm", bufs=4))
        og_pool = ctx.enter_context(tc.tile_pool(name="og", bufs=2))

        # per-slice data tiles (separate tiles => precise dependency ranges)
        xT_sb = [dat.tile([128, EC, 512], BF16, tag=f"xT{t}", name=f"xT{t}")
                 for t in range(4)]
        qf8 = [[dat.tile([128, 512], F8, tag=f"qf8{i}_{t}", name=f"qf8{i}_{t}")
                for t in range(4)] for i in range(2)]
        kf8 = [[dat.tile([128, 512], F8, tag=f"kf8{i}_{t}", name=f"kf8{i}_{t}")
                for t in range(4)] for i in range(2)]
        qT8 = [[dat.tile([64, 2, 512], F8, tag=f"qT8{i}_{t}", name=f"qT8{i}_{t}")
                for t in range(4)] for i in range(2)]
        kT8 = [[dat.tile([64, 2, 512], F8, tag=f"kT8{i}_{t}", name=f"kT8{i}_{t}")
                for t in range(4)] for i in range(2)]
        v_sb = [dat.tile([128, 4, HPC, 65], BF16, tag=f"v{t}", name=f"v{t}")
                for t in range(4)]
        attT = [[dat.tile([128, 512], BF16, tag=f"attT{i}_{t}", name=f"attT{i}_{t}")
                 for t in range(4)] for i in range(2)]
        wq8_sb = dat.tile([128, EC, DP], F8, tag="wq8", name="wq8_sb")
        wk_lo = dat.tile([128, 4, DP], BF16, tag="wk_lo", name="wk_lo")
        wk_hi = dat.tile([128, 4, DP], BF16, tag="wk_hi", name="wk_hi")
        xT8_sb = [dat.tile([128, EC, 512], F8, tag=f"xT8{t}", name=f"xT8{t}")
                  for t in range(4)]
        wv_sb = dat.tile([128, EC, DP], BF16, tag="wv", name="wv_sb")
        wo_sb = dat.tile([128, 2, E], BF16, tag="wo", name="wo_sb")
        tri_sb = dat.tile([128, 128], BF16, tag="tri", name="tri_sb")

        nc.gpsimd.load_library(library_config.attn)
        # PE p-state warmup: a dummy matmul at t~0 starts the frequency
        # ramp so the first real chains run at full speed
        wa = dat.tile([1, 256], BF16, tag="wa", name="wa")
        nc.vector.memset(wa[:], 0.0)
        wps = po_pool.tile([128, 512], F32, tag="po", name="wps")
        nc.tensor.matmul(wps[0:1, 0:256], wa[0:1, 0:1], wa[:],
                         start=True, stop=True)
        # warm the exp ACT table set from a memset scratch (no DMA dep), so
        # the ~1.3us ACT_TABLE_LOAD is off the attention exp stream
        warm = dat.tile([1, 1], F32, tag="warm", name="warm")
        nc.vector.memset(warm[:], 0.0)
        nc.scalar.activation(warm[:], warm[:], AF.Exp, bias=0.0, scale=1.0)
        # rowsum ones columns: memset on DVE, no scattered DMA
        for t in range(4):
            nc.vector.memset(v_sb[t][:, :, :, 64:65], 1.0)

        # ---- input DMA, ordered for earliest first projection chain ----
        xTr = xT_d.rearrange("(c p) s -> p c s", p=128)
        xT8r = xT8_d.rearrange("(c p) s -> p c s", p=128)
        wkr = wk_d.rearrange("(c p) d -> p c d", p=128)
        nc.sync.dma_start(wk_lo[:], wkr[:, 0:4, :])
        nc.sync.dma_start(xT_sb[0][:, 0:4, :], xTr[:, 0:4, 0:512])
        nc.sync.dma_start(wk_hi[:], wkr[:, 4:8, :])
        nc.sync.dma_start(xT_sb[0][:, 4:8, :], xTr[:, 4:8, 0:512])
        nc.sync.dma_start(wv_sb[:], wv_d.rearrange("(c p) d -> p c d", p=128))
        nc.sync.dma_start(wq8_sb[:], wq_d.rearrange("(c p) d -> p c d", p=128))
        nc.sync.dma_start(xT8_sb[0][:], xT8r[:, :, 0:512])
        nc.sync.dma_start(xT_sb[1][:], xTr[:, :, 512:1024])
        nc.sync.dma_start(xT8_sb[1][:], xT8r[:, :, 512:1024])
        nc.sync.dma_start(tri_sb[:], tri_d[:, :])

        def late_inputs():
            # emitted after the slice-0 projection units so their transfers
            # queue BEHIND the first attention block's shuffle DMAs
            nc.sync.dma_start(xT_sb[2][:], xTr[:, :, 1024:1536])
            nc.sync.dma_start(xT8_sb[2][:], xT8r[:, :, 1024:1536])
            nc.sync.dma_start(wo_sb[:], wo_d.rearrange("(c p) e -> p c e", p=128))
            nc.sync.dma_start(xT_sb[3][:], xTr[:, :, 1536:2048])
            nc.sync.dma_start(xT8_sb[3][:], xT8r[:, :, 1536:2048])

        oTr = oT_d.rearrange("(a p) s -> p a s", p=128)

        # ---- unit emitters (PE work packets fed through the drip queue) ----
        def stage_copy(dst, src, act=False):
            """PSUM->SBUF staging (DVE or ACT: GPSIMD cannot access PSUM)."""
            if act:
                nc.scalar.copy(dst, src)
            else:
                nc.vector.tensor_copy(dst, src)

        def qk_unit(t, which, hp):
            """projection chain: q or k, head pair hp, s-slice t.  The host
            permutes w_q/w_k columns to m = 64*dh + 32*p + dl (d = 32*dh+dl,
            head parity p), so the fp8 copy's rows regroup into the
            DoubleRow [dl, dh, s] layout with two contiguous-partition
            shuffle DMAs."""
            ps = po_pool.tile([128, 512], F32, tag="po", name="ps_qk")
            if which == "q":
                # fp8 DoubleRow over e-chunk pairs: 4 matmuls, 2x rows each
                for j in range(EC // 2):
                    nc.tensor.matmul(
                        ps[:],
                        wq8_sb[:, 2 * j:2 * j + 2, 128 * hp:128 * hp + 128],
                        xT8_sb[t][:, 2 * j:2 * j + 2, :],
                        start=(j == 0), stop=(j == EC // 2 - 1),
                        perf_mode=DR,
                    )
            else:
                for ec in range(EC):
                    w = wk_lo if ec < 4 else wk_hi
                    nc.tensor.matmul(
                        ps[:],
                        w[:, ec % 4, 128 * hp:128 * hp + 128],
                        xT_sb[t][:, ec, :],
                        start=(ec == 0), stop=(ec == EC - 1),
                    )
            f8 = (qf8 if which == "q" else kf8)[hp][t]
            d8 = (qT8 if which == "q" else kT8)[hp][t]
            nc.vector.tensor_copy(f8[:], ps[:])
            for dh in range(2):
                nc.sync.dma_start(d8[:, dh, :], f8[64 * dh:64 * dh + 64, :])

        def v_unit(sc):
            """projection chain: v columns for s-chunk sc."""
            ps = po_pool.tile([128, 512], F32, tag="po", name="ps_v")
            for ec in range(EC):
                nc.tensor.matmul(
                    ps[:, 0:256],
                    xT_sb[sc // 4][:, ec, 128 * (sc % 4):128 * (sc % 4) + 128],
                    wv_sb[:, ec, :],
                    start=(ec == 0), stop=(ec == EC - 1),
                )
            nc.vector.tensor_copy(
                v_sb[sc // 4][:, sc % 4, :, 0:64],
                ps[:, 0:256].rearrange("p (h d) -> p h d", h=HPC))

        og_tiles = {}
        units_done = {}
        copy_pend = []

        def oproj_mms(t, et, h=None):
            """matmul half of an oproj unit (pure PE filler)."""
            if h is None:
                o, n, off = 512 * t, 512, 0
            else:
                o, n, off = 512 * t + 256 * h, 256, 256 * h
            ps = po_pool.tile([128, 512], F32, tag="po", name="ps_po")
            for hp in range(2):
                nc.tensor.matmul(
                    ps[:, 0:n],
                    wo_sb[:, hp, 128 * et:128 * et + 128],
                    attT[hp][t][:, off:off + n],
                    start=(hp == 0), stop=(hp == 1),
                )
            copy_pend.append((ps, t, et, h))

        def oproj_copy():
            ps, t, et, h = copy_pend.pop(0)
            if h is None:
                o, n, off, key = 512 * t, 512, 0, t
            else:
                o, n, off, key = 512 * t + 256 * h, 256, 256 * h, (t, h)
            og = og_tiles[t]
            stage_copy(og[:, et, off:off + n], ps[:, 0:n])
            units_done[key] = units_done.get(key, 0) + 1
            if h == 1:
                # tail: two half-batches so the last DMA chain is short
                if units_done[key] == 4:
                    nc.sync.dma_start(oTr[:, 0:4, o:o + n], og[:, 0:4, off:off + n])
                elif units_done[key] == EC:
                    nc.sync.dma_start(oTr[:, 4:8, o:o + n], og[:, 4:8, off:off + n])
            elif units_done[key] == EC:
                nc.sync.dma_start(oTr[:, :, o:o + n], og[:, :, off:off + n])

        def oproj_unit(t, et, h=None):
            oproj_mms(t, et, h)
            oproj_copy()

        def drip_op_mms():
            """mid-block PE filler: emit an oproj matmul pair if a PSUM
            chain buffer is free (its copy lands at the next fin slot).
            Only taps the queue when enough units remain for the fin
            boundary fills."""
            if len(copy_pend) < 2 and len(queue) > 8:
                for i, (kind, args) in enumerate(queue):
                    if kind == "op":
                        queue.pop(i)
                        oproj_mms(*args)
                        return

        queue = []

        def drip(reserve=0):
            if len(queue) > reserve:
                kind, args = queue.pop(0)
                UNIT_FNS[kind](*args)

        def flush(pred=lambda u: True):
            rest = []
            for u in queue:
                if pred(u):
                    UNIT_FNS[u[0]](*u[1])
                else:
                    rest.append(u)
            queue[:] = rest

        UNIT_FNS = {"qk": qk_unit, "v": v_unit, "op": oproj_unit}

        # ---- software-pipelined attention ----
        deferred = [None]

        def attention_block(hp, qb):
            """kv loop for head pair hp, query block qb: scores run one
            group ahead of av; last av + normalize deferred to next block."""
            t = qb // 2
            q0 = 256 * (qb % 2)
            ncols = 2 * (qb + 1)
            G = ncols // GRP
            outp = ou_pool.tile([65, 512], F32, tag="ou", name="outp")
            # both parities share one PSUM bank: the first matmul's
            # start=True clears has_written bank-wide, so parity 1's first
            # chunk (start=False) lands on pending-zero bytes
            stps, ptiles = [], []

            def scores(gi):
                stp = st_pool.tile([128, 512 * GRP], F32, tag="st", name="stp")
                for i, c in enumerate(range(GRP * gi, GRP * gi + GRP)):
                    for p in range(2):
                        slot = i if p == 0 else GRP + (i + 1) % GRP
                        nc.tensor.matmul(
                            stp[:, 256 * slot:256 * slot + 256],
                            kT8[hp][c // 4][32 * p:32 * p + 32, :,
                                            128 * (c % 4):128 * (c % 4) + 128],
                            qT8[hp][t][32 * p:32 * p + 32, :, q0:q0 + SQ],
                            start=True, stop=True, perf_mode=DR,
                        )
                stps.append(stp)

            def exp_mask(gi):
                ptile = pt_pool.tile([128, 512 * GRP], BF16, tag="pt",
                                     name="ptile")
                nc.scalar.activation(ptile[:], stps[-1][:], AF.Exp,
                                     bias=0.0, scale=0.125)
                if gi == qb:  # group holding the diagonal chunks
                    for i in range(2):
                        for p in range(2):
                            slot = i if p == 0 else GRP + (i + 1) % GRP
                            sl_ = ptile[:, 256 * slot:256 * slot + 256]
                            if i == 0:
                                nc.vector.tensor_mul(
                                    sl_[:, 0:128], sl_[:, 0:128], tri_sb[:])
                            else:
                                nc.vector.tensor_mul(
                                    sl_[:, 128:256], sl_[:, 128:256], tri_sb[:])
                ptiles.append(ptile)

            # groups processed diagonal-first: the mask chain leaves the
            # block-end critical path and the rowsum still sums all chunks
            order = [G - 1] + list(range(G - 1))
            last_gi = order[-1]

            def av(gi):
                first_av = gi == G - 1
                for i, c in enumerate(range(GRP * gi, GRP * gi + GRP)):
                    for p in range(2):
                        slot = i if p == 0 else GRP + (i + 1) % GRP
                        h = 2 * hp + p
                        if c == 2 * qb + 1:
                            rs, n = 128, 128   # left half fully masked
                        else:
                            rs, n = 0, 256
                        nc.tensor.matmul(
                            outp[:, 256 * p + rs:256 * p + rs + n],
                            v_sb[c // 4][:, c % 4, h, :],
                            ptiles[ptmap[gi]][:, 256 * slot + rs:256 * slot + rs + n],
                            start=(first_av and i == 0 and p == 0),
                            stop=(gi == last_gi and i == GRP - 1),
                            skip_group_check=True,
                        )

            av_of = av
            scores(order[0])
            if deferred[0] is not None:
                deferred[0]()
                deferred[0] = None
            ptmap = {}
            for oi, gi in enumerate(order):
                exp_mask(gi)
                ptmap[gi] = len(ptiles) - 1
                if oi + 1 < G:
                    scores(order[oi + 1])
                if oi >= 1:
                    av_of(order[oi - 1])

            def fin():
                av_of(order[-1])
                # normalize: 1/rowsum, broadcast to 64 partitions, multiply
                # into attT.  The FINAL block's chain is on the kernel-exit
                # critical path, so it runs split per parity (recip half ->
                # broadcast half -> mul pipelines across DVE/Pool).
                recip = nrm_pool.tile([1, 512], F32, tag="recip", name="recip")
                recipb = nrm_pool.tile([64, 512], F32, tag="recipb",
                                       name="recipb")
                if hp == 1 and qb == 7:
                    for p in range(2):
                        sl_ = slice(256 * p, 256 * p + 256)
                        nc.vector.reciprocal(recip[:, sl_], outp[64:65, sl_])
                        nc.gpsimd.partition_broadcast(
                            recipb[:, sl_], recip[0:1, sl_], channels=64)
                    for p in range(2):
                        sl_ = slice(256 * p, 256 * p + 256)
                        nc.vector.tensor_mul(
                            attT[hp][t][64 * p:64 * p + 64, q0:q0 + SQ],
                            outp[0:64, sl_], recipb[0:64, sl_])
                else:
                    nc.vector.reciprocal(recip[:], outp[64:65, :])
                    nc.gpsimd.partition_broadcast(recipb[:], recip[0:1, :],
                                                  channels=64)
                    for p in range(2):
                        nc.vector.tensor_mul(
                            attT[hp][t][64 * p:64 * p + 64, q0:q0 + SQ],
                            outp[0:64, 256 * p:256 * p + 256],
                            recipb[0:64, 256 * p:256 * p + 256])
                while copy_pend:
                    oproj_copy()
                drip()
                drip()

            deferred[0] = fin

        # ---- main schedule ----
        # slice-0 projections run directly (pipeline fill); k first (its
        # inputs land first), q (fp8, cheap) last
        for hp in range(2):
            qk_unit(0, "k", hp)
        for sc in range(4):
            v_unit(sc)
        for hp in range(2):
            qk_unit(0, "q", hp)
        late_inputs()
        # slice-1 k chains fill the window while slice-0's q/k shuffle DMAs
        # land (first attention block waits on those)
        for hp in range(2):
            qk_unit(1, "k", hp)

        for t in range(4):
            og_tiles[t] = og_pool.tile([128, EC, 512], BF16, tag="og",
                                       name=f"og{t}")
            if t < 3:
                # projections for the next slice drip during this stretch;
                # they go to the queue FRONT (hard deadline at the stretch
                # boundary, unlike the leftover oproj units)
                front = []
                for hp in range(2):
                    front.append(("qk", (t + 1, "q", hp)))
                    if t > 0:
                        front.append(("qk", (t + 1, "k", hp)))
                for sc in range(4 * t + 4, 4 * t + 8):
                    front.append(("v", (sc,)))
                queue[0:0] = front
            for qb in (2 * t, 2 * t + 1):
                for hp in range(2):
                    attention_block(hp, qb)
                if t == 3 and qb == 6:
                    # first half of slice 3; the qb=7 half is handled by the
                    # hp-split tail units
                    queue.extend(("op", (3, et, 0)) for et in range(EC))
            # boundary: next-slice projections must be emitted before the
            # next stretch's first block references them
            flush(lambda u: u[0] in ("qk", "v"))
            # oproj units for this stretch drip during the next one
            if t < 3:
                queue.extend(("op", (t, et)) for et in range(EC))

        # the 8 units for the last half-slice (t=3, h=1) are hp-split:
        # hp0 matmuls run while the final normalize chain computes, hp1
        # matmuls + staging + DMA form the only work after it.  Four units
        # pack per [128,1024] scores-pool tile; within each 2KB bank the
        # first matmul uses start=True (bank-wide pending-zero), the second
        # start=False lands on pending-zero bytes.
        tail_tiles = []

        def tail_hp0():
            for i in range(2):
                tail_tiles.append(st_pool.tile([128, 1024], F32, tag="st",
                                               name=f"fin{i}"))
            for et in range(EC):
                ps = tail_tiles[et // 4][:, 256 * (et % 4):256 * (et % 4) + 256]
                nc.tensor.matmul(
                    ps, wo_sb[:, 0, 128 * et:128 * et + 128],
                    attT[0][3][:, 256:512],
                    start=(et % 2 == 0), stop=False, skip_group_check=True,
                )

        def tail_hp1():
            og = og_tiles[3]
            for et in range(EC):
                ps = tail_tiles[et // 4][:, 256 * (et % 4):256 * (et % 4) + 256]
                nc.tensor.matmul(
                    ps, wo_sb[:, 1, 128 * et:128 * et + 128],
                    attT[1][3][:, 256:512],
                    start=False, stop=True, skip_group_check=True,
                )
            for pair in range(4):
                ps = tail_tiles[pair // 2][:, 512 * (pair % 2):512 * (pair % 2) + 512]
                stage_copy(og[:, 2 * pair:2 * pair + 2, 256:512],
                           ps.rearrange("p (u s) -> p u s", u=2),
                           act=(pair % 2 == 1))
                if pair == 1:
                    nc.sync.dma_start(oTr[:, 0:4, 1792:2048], og[:, 0:4, 256:512])
                elif pair == 2:
                    nc.sync.dma_start(oTr[:, 4:6, 1792:2048], og[:, 4:6, 256:512])
                elif pair == 3:
                    nc.sync.dma_start(oTr[:, 6:8, 1792:2048], og[:, 6:8, 256:512])

        # drain any queued units first (PE filler while the last exp runs),
        # then the hp0 halves, the deferred final block, and the hp1 halves
        queue[:] = [u for u in queue if u[1][0] != 3 or u[1][2] != 1]
        for kind, args in list(queue):
            oproj_unit(*args)
        queue.clear()
        while copy_pend:
            oproj_copy()
        deferred[0]()
        deferred[0] = None
        # hp0 halves emitted after the final fin: they run on PE during the
        # normalize chain (recip/broadcast/muls on DVE+Pool), which otherwise
        # has no PE work at all
        tail_hp0()
        tail_hp1()

    nc.compile()
    return nc


_NC_CACHE = None
_LAST_IN_MAPS = None


def kernel(x, w_q, w_k, w_v, w_o):
    global _NC_CACHE, _LAST_IN_MAPS
    if _NC_CACHE is None:
        _NC_CACHE = build_kernel()
    nc = _NC_CACHE

    bf = ml_dtypes.bfloat16
    x = np.asarray(x, dtype=np.float32)
    w_q = np.asarray(w_q, dtype=np.float32)
    w_k = np.asarray(w_k, dtype=np.float32)
    w_v = np.asarray(w_v, dtype=np.float32)
    w_o = np.asarray(w_o, dtype=np.float32)

    tri = make_tri()
    # within each head pair's 128 columns, m = 64*dh + 32*p + dl maps to
    # the original column 64*p + 32*dh + dl (d = 32*dh + dl)
    perm128 = np.array([64 * ((m % 64) // 32) + 32 * (m // 64) + (m % 32)
                        for m in range(128)])
    perm = np.concatenate([128 * i + perm128 for i in range(2)])
    in_maps = []
    for core in range(NCORES):
        b, g = divmod(core, NCORES // B)
        sl = slice(g * DP, (g + 1) * DP)
        in_maps.append({
            "xT": np.ascontiguousarray(x[b].T).astype(bf),
            "xT8": np.ascontiguousarray(x[b].T).astype(ml_dtypes.float8_e4m3),
            "wq": np.ascontiguousarray(w_q[:, sl][:, perm]).astype(
                ml_dtypes.float8_e4m3),
            "wk": np.ascontiguousarray(w_k[:, sl][:, perm]).astype(bf),
            "wv": np.ascontiguousarray(w_v[:, sl]).astype(bf),
            "wo": np.ascontiguousarray(w_o[sl, :]).astype(bf),
            "tri": tri,
        })

    _LAST_IN_MAPS = in_maps
    res = bass_utils.run_bass_kernel_spmd(nc, in_maps, core_ids=list(range(NCORES)))

    out = np.zeros((B, S, E), dtype=np.float32)
    for core in range(NCORES):
        b = core // (NCORES // B)
        out[b] += res.results[core]["oT"].T.astype(np.float32)
    return out
